# revision 9
# baseline (speedup 1.0000x reference)
# MinGRU block kernel for 8 Trainium2 NeuronCores (Bass/Tile).
#
# Reference computation (B=4, L=8192, D=1024, f32):
#   norm = rmsnorm(inp, ln_w)
#   beta = sigmoid(norm @ Wg.T); hx_hat = norm @ Wc.T
#   a = 1-beta; x = beta*hx_hat
#   h = assoc_scan(h_t = a_t*h_{t-1} + x_t) along L
#   out = h + SwiGLU_FFN(rmsnorm(h, ffn_w));  returns (out, h)
#
# Sharding: 8 cores = 4 batches x 2 sequence halves. The scan carry between
# the two halves of a batch is exchanged on host between two launches:
#   L1: rmsnorm + gate matmuls (bf16) + local scan -> a,x (bf16, DRAM), h_last
#   L2: scan(a, x, initial=carry) + FFN + residual -> out^T, hx^T (host
#       transposes back; everything on-device stays in [channel, token]
#       layout so no PE transposes are needed in L2)
# ln_w / ffn_w are folded into the matmul weights on host (exact).

import sys

sys.path.insert(0, "/opt/trn_rl_repo")

import numpy as np
import ml_dtypes

import concourse.bass as bass
import concourse.tile as tile
from concourse import mybir, bacc, masks
from concourse.bass_utils import run_bass_kernel_spmd

B, L, D = 4, 8192, 1024
NCORES = 8
T = L // 2        # tokens per core
TT = 512          # token tile
NT = T // TT      # 8 token tiles per core
NSUB = TT // 128  # 4 norm sub-tiles per token tile
KC = D // 128     # contraction chunks
EC = D // 128     # output-channel chunks
EPS = 1e-6

f32 = mybir.dt.float32
bf16 = mybir.dt.bfloat16
AF = mybir.ActivationFunctionType
OP = mybir.AluOpType
bf16_np = ml_dtypes.bfloat16


def build_l1():
    nc = bacc.Bacc(None, target_bir_lowering=False)
    inp_s = nc.dram_tensor("inp_s", [T, D], f32, kind="ExternalInput")
    wgT_d = nc.dram_tensor("wgT", [D, D], bf16, kind="ExternalInput")
    wcT_d = nc.dram_tensor("wcT", [D, D], bf16, kind="ExternalInput")
    a_T = nc.dram_tensor("a_T", [D, T], bf16, kind="ExternalOutput")
    x_T = nc.dram_tensor("x_T", [D, T], bf16, kind="ExternalOutput")
    hlast = nc.dram_tensor("hlast", [128, EC], f32, kind="ExternalOutput")

    with tile.TileContext(nc) as tc:
        with (
            tc.tile_pool(name="wpool", bufs=1) as wpool,
            tc.tile_pool(name="xin", bufs=4) as xin,
            tc.tile_pool(name="norm", bufs=3) as norm,
            tc.tile_pool(name="xnt", bufs=2) as xntp,
            tc.tile_pool(name="gate", bufs=4) as gate,
            tc.tile_pool(name="scan", bufs=2) as scanp,
            tc.tile_pool(name="per", bufs=1) as per,
            tc.tile_pool(name="psum_mm", bufs=2, space=bass.MemorySpace.PSUM) as psum_mm,
            tc.tile_pool(name="psum_tr", bufs=4, space=bass.MemorySpace.PSUM) as psum_tr,
        ):
            ident = per.tile([128, 128], bf16)
            masks.make_identity(nc, ident[:])
            hprev = per.tile([128, EC], f32)
            nc.vector.memset(hprev[:], 0.0)
            eps_t = per.tile([128, 1], f32)
            nc.vector.memset(eps_t[:], EPS)

            wg_sb = wpool.tile([128, KC, D], bf16)
            nc.sync.dma_start(wg_sb[:], wgT_d[:].rearrange("(k p) e -> p k e", p=128))
            wc_sb = wpool.tile([128, KC, D], bf16)
            nc.sync.dma_start(wc_sb[:], wcT_d[:].rearrange("(k p) e -> p k e", p=128))

            for i in range(NT):
                # --- rmsnorm + transpose into [d, t] layout (bf16) ---
                xnt = xntp.tile([128, KC, TT], bf16)
                for s in range(NSUB):
                    t0 = i * TT + s * 128
                    xt = xin.tile([128, D], f32)
                    nc.sync.dma_start(xt[:], inp_s[t0 : t0 + 128, :])
                    sq = norm.tile([128, D], bf16, tag="sq")
                    ssq = norm.tile([128, 1], f32, tag="ssq")
                    nc.scalar.activation(sq[:], xt[:], AF.Square, accum_out=ssq[:])
                    rms = norm.tile([128, 1], f32, tag="rms")
                    nc.scalar.activation(
                        rms[:], ssq[:], AF.Sqrt, scale=1.0 / D, bias=eps_t[:]
                    )
                    inv = norm.tile([128, 1], f32, tag="inv")
                    nc.vector.reciprocal(inv[:], rms[:])
                    xn = norm.tile([128, D], bf16, tag="xn")
                    nc.vector.tensor_scalar_mul(xn[:], xt[:], inv[:])
                    for k in range(KC):
                        ptr = psum_tr.tile([128, 128], bf16)
                        nc.tensor.transpose(
                            ptr[:], xn[:, k * 128 : (k + 1) * 128], ident[:]
                        )
                        nc.vector.tensor_copy(
                            xnt[:, k, s * 128 : (s + 1) * 128], ptr[:]
                        )

                # --- gate matmuls + elementwise + local scan ---
                for e in range(EC):
                    pm_g = psum_mm.tile([128, TT], f32, tag="pmg")
                    for k in range(KC):
                        nc.tensor.matmul(
                            pm_g[:],
                            wg_sb[:, k, e * 128 : (e + 1) * 128],
                            xnt[:, k, :],
                            start=(k == 0),
                            stop=(k == KC - 1),
                        )
                    beta = gate.tile([128, TT], f32, tag="beta")
                    nc.scalar.activation(beta[:], pm_g[:], AF.Sigmoid)

                    pm_c = psum_mm.tile([128, TT], f32, tag="pmc")
                    for k in range(KC):
                        nc.tensor.matmul(
                            pm_c[:],
                            wc_sb[:, k, e * 128 : (e + 1) * 128],
                            xnt[:, k, :],
                            start=(k == 0),
                            stop=(k == KC - 1),
                        )
                    at = gate.tile([128, TT], bf16, tag="at")
                    nc.vector.tensor_scalar(at[:], beta[:], -1.0, 1.0, OP.mult, OP.add)
                    xv = gate.tile([128, TT], bf16, tag="xv")
                    nc.vector.tensor_mul(xv[:], beta[:], pm_c[:])
                    nc.sync.dma_start(
                        a_T[e * 128 : (e + 1) * 128, i * TT : (i + 1) * TT], at[:]
                    )
                    nc.sync.dma_start(
                        x_T[e * 128 : (e + 1) * 128, i * TT : (i + 1) * TT], xv[:]
                    )
                    h = scanp.tile([128, TT], f32)
                    nc.vector.tensor_tensor_scan(
                        h[:], at[:], xv[:], hprev[:, e : e + 1], OP.mult, OP.add
                    )
                    nc.vector.tensor_copy(hprev[:, e : e + 1], h[:, TT - 1 : TT])

            nc.sync.dma_start(hlast[:], hprev[:])

    nc.compile()
    return nc


def build_l2():
    nc = bacc.Bacc(None, target_bir_lowering=False)
    a_in = nc.dram_tensor("a_in", [D, T], bf16, kind="ExternalInput")
    x_in = nc.dram_tensor("x_in", [D, T], bf16, kind="ExternalInput")
    carry_d = nc.dram_tensor("carry", [128, EC], f32, kind="ExternalInput")
    w1T_d = nc.dram_tensor("w1T", [D, D], bf16, kind="ExternalInput")
    w3T_d = nc.dram_tensor("w3T", [D, D], bf16, kind="ExternalInput")
    w2T_d = nc.dram_tensor("w2T", [D, D], bf16, kind="ExternalInput")
    out_sT = nc.dram_tensor("out_sT", [D, T], f32, kind="ExternalOutput")
    hx_sT = nc.dram_tensor("hx_sT", [D, T], f32, kind="ExternalOutput")

    with tile.TileContext(nc) as tc:
        with (
            tc.tile_pool(name="wpool", bufs=1) as wpool,
            tc.tile_pool(name="ax", bufs=4) as ax,
            tc.tile_pool(name="scan", bufs=2) as scanp,
            tc.tile_pool(name="nrm", bufs=2) as nrm,
            tc.tile_pool(name="hn", bufs=2) as hnp,
            tc.tile_pool(name="ffn", bufs=2) as ffn,
            tc.tile_pool(name="outp", bufs=4) as outp,
            tc.tile_pool(name="per", bufs=1) as per,
            tc.tile_pool(name="psum_mm", bufs=2, space=bass.MemorySpace.PSUM) as psum_mm,
            tc.tile_pool(name="psum_s", bufs=2, space=bass.MemorySpace.PSUM) as psum_s,
            tc.tile_pool(name="psum_o", bufs=2, space=bass.MemorySpace.PSUM) as psum_o,
        ):
            hprev = per.tile([128, EC], f32)
            carry_sb = per.tile([128, EC], f32)
            nc.sync.dma_start(carry_sb[:], carry_d[:])
            eps_row = per.tile([1, 1], f32)
            nc.vector.memset(eps_row[:], EPS)
            ones_b = per.tile([128, 1], bf16)
            nc.vector.memset(ones_b[:], 1.0)

            w1_sb = wpool.tile([128, KC, D], bf16)
            nc.sync.dma_start(w1_sb[:], w1T_d[:].rearrange("(k p) e -> p k e", p=128))
            w3_sb = wpool.tile([128, KC, D], bf16)
            nc.sync.dma_start(w3_sb[:], w3T_d[:].rearrange("(k p) e -> p k e", p=128))
            w2_sb = wpool.tile([128, KC, D], bf16)
            nc.sync.dma_start(w2_sb[:], w2T_d[:].rearrange("(k p) e -> p k e", p=128))

            for i in range(NT):
                # --- scan + hx store + sum-of-squares accumulation (PE) ---
                hs = []
                pm_ssq = psum_s.tile([1, TT], f32)
                for c in range(EC):
                    at = ax.tile([128, TT], bf16, tag="a")
                    nc.sync.dma_start(
                        at[:], a_in[c * 128 : (c + 1) * 128, i * TT : (i + 1) * TT]
                    )
                    xv = ax.tile([128, TT], bf16, tag="x")
                    nc.sync.dma_start(
                        xv[:], x_in[c * 128 : (c + 1) * 128, i * TT : (i + 1) * TT]
                    )
                    h = scanp.tile([128, TT], f32, tag=f"h{c}")
                    init = carry_sb[:, c : c + 1] if i == 0 else hprev[:, c : c + 1]
                    nc.vector.tensor_tensor_scan(
                        h[:], at[:], xv[:], init, OP.mult, OP.add
                    )
                    nc.vector.tensor_copy(hprev[:, c : c + 1], h[:, TT - 1 : TT])
                    nc.sync.dma_start(
                        hx_sT[c * 128 : (c + 1) * 128, i * TT : (i + 1) * TT], h[:]
                    )
                    hsq = nrm.tile([128, TT], bf16, tag="hsq")
                    nc.scalar.square(hsq[:], h[:])
                    nc.tensor.matmul(
                        pm_ssq[:],
                        ones_b[:],
                        hsq[:],
                        start=(c == 0),
                        stop=(c == EC - 1),
                    )
                    hs.append(h)

                # --- rms row -> inv -> broadcast -> hn^T (bf16) ---
                rms_row = nrm.tile([1, TT], f32, tag="rmsrow")
                nc.scalar.activation(
                    rms_row[:], pm_ssq[:], AF.Sqrt, scale=1.0 / D, bias=eps_row[:]
                )
                inv_row = nrm.tile([1, TT], f32, tag="invrow")
                nc.vector.reciprocal(inv_row[:], rms_row[:])
                inv_bc = nrm.tile([128, TT], f32, tag="invbc")
                nc.gpsimd.partition_broadcast(inv_bc[:], inv_row[:])
                hnT = []
                for c in range(EC):
                    hn_c = hnp.tile([128, TT], bf16, tag=f"hn{c}")
                    nc.vector.tensor_mul(hn_c[:], hs[c][:], inv_bc[:])
                    hnT.append(hn_c)

                # --- FFN: u = silu(w1 @ hn) * (w3 @ hn) ---
                us = []
                for e in range(EC):
                    pm_g = psum_mm.tile([128, TT], f32, tag="pmg")
                    for k in range(KC):
                        nc.tensor.matmul(
                            pm_g[:],
                            w1_sb[:, k, e * 128 : (e + 1) * 128],
                            hnT[k][:],
                            start=(k == 0),
                            stop=(k == KC - 1),
                        )
                    sil = ffn.tile([128, TT], bf16, tag="sil")
                    nc.scalar.activation(sil[:], pm_g[:], AF.Silu)
                    pm_r = psum_mm.tile([128, TT], f32, tag="pmr")
                    for k in range(KC):
                        nc.tensor.matmul(
                            pm_r[:],
                            w3_sb[:, k, e * 128 : (e + 1) * 128],
                            hnT[k][:],
                            start=(k == 0),
                            stop=(k == KC - 1),
                        )
                    u_e = ffn.tile([128, TT], bf16, tag=f"u{e}")
                    nc.vector.tensor_mul(u_e[:], sil[:], pm_r[:])
                    us.append(u_e)

                # --- ff^T = w2 @ u, residual in [d, t], store ---
                for e in range(EC):
                    pm_f = psum_o.tile([128, TT], f32)
                    for hc in range(KC):
                        nc.tensor.matmul(
                            pm_f[:],
                            w2_sb[:, hc, e * 128 : (e + 1) * 128],
                            us[hc][:],
                            start=(hc == 0),
                            stop=(hc == KC - 1),
                        )
                    outf = outp.tile([128, TT], f32)
                    nc.vector.tensor_add(outf[:], pm_f[:], hs[e][:])
                    nc.sync.dma_start(
                        out_sT[e * 128 : (e + 1) * 128, i * TT : (i + 1) * TT], outf[:]
                    )

    nc.compile()
    return nc


_CACHE = {}
last_perf = {}


def _get_programs():
    if "l1" not in _CACHE:
        _CACHE["l1"] = build_l1()
        _CACHE["l2"] = build_l2()
    return _CACHE["l1"], _CACHE["l2"]


def kernel(inp, Wg, Wc, w1, w2, w3, ln_w, ffn_w):
    import os
    import time

    trace = bool(int(os.environ.get("MINGRU_TRACE", "0")))
    nc1, nc2 = _get_programs()

    inp = np.asarray(inp, np.float32)
    # fold the norm scales into the following matmuls (exact)
    wgT = np.ascontiguousarray((np.asarray(Wg, np.float32) * np.asarray(ln_w, np.float32)).T).astype(bf16_np)
    wcT = np.ascontiguousarray((np.asarray(Wc, np.float32) * np.asarray(ln_w, np.float32)).T).astype(bf16_np)
    w1T = np.ascontiguousarray((np.asarray(w1, np.float32) * np.asarray(ffn_w, np.float32)).T).astype(bf16_np)
    w3T = np.ascontiguousarray((np.asarray(w3, np.float32) * np.asarray(ffn_w, np.float32)).T).astype(bf16_np)
    w2T = np.ascontiguousarray(np.asarray(w2, np.float32).T).astype(bf16_np)

    in1 = []
    for c in range(NCORES):
        b, half = divmod(c, 2)
        in1.append(
            {
                "inp_s": np.ascontiguousarray(inp[b, half * T : (half + 1) * T, :]),
                "wgT": wgT,
                "wcT": wcT,
            }
        )
    t0 = time.time()
    r1 = run_bass_kernel_spmd(nc1, in1, core_ids=list(range(NCORES)), trace=trace)
    t1 = time.time()

    zeros = np.zeros((128, EC), np.float32)
    in2 = []
    for c in range(NCORES):
        b, half = divmod(c, 2)
        carry = r1.results[2 * b]["hlast"] if half == 1 else zeros
        in2.append(
            {
                "a_in": r1.results[c]["a_T"],
                "x_in": r1.results[c]["x_T"],
                "carry": np.ascontiguousarray(carry),
                "w1T": w1T,
                "w3T": w3T,
                "w2T": w2T,
            }
        )
    t2 = time.time()
    r2 = run_bass_kernel_spmd(nc2, in2, core_ids=list(range(NCORES)), trace=trace)
    t3 = time.time()

    out = np.empty((B, L, D), np.float32)
    hx = np.empty((B, L, D), np.float32)
    for c in range(NCORES):
        b, half = divmod(c, 2)
        out[b, half * T : (half + 1) * T, :] = r2.results[c]["out_sT"].T
        hx[b, half * T : (half + 1) * T, :] = r2.results[c]["hx_sT"].T

    last_perf["r1"] = r1
    last_perf["r2"] = r2
    last_perf["t_l1"] = t1 - t0
    last_perf["t_l2"] = t3 - t2
    return out, hx


# revision 11
# speedup vs baseline: 1.2849x; 1.2849x over previous
# MinGRU block kernel for 8 Trainium2 NeuronCores (Bass/Tile).
#
# Reference computation (B=4, L=8192, D=1024, f32):
#   norm = rmsnorm(inp, ln_w)
#   beta = sigmoid(norm @ Wg.T); hx_hat = norm @ Wc.T
#   a = 1-beta; x = beta*hx_hat
#   h = assoc_scan(h_t = a_t*h_{t-1} + x_t) along L
#   out = h + SwiGLU_FFN(rmsnorm(h, ffn_w));  returns (out, h)
#
# Sharding: 8 cores = 4 batches x 2 sequence halves. The scan carry between
# the two halves of a batch is exchanged on host between two launches:
#   L1: rmsnorm + gate matmuls (bf16) + local scan -> a,x (bf16, DRAM), h_last
#   L2: scan(a, x, initial=carry) + FFN + residual -> out^T, hx^T (host
#       transposes back; everything on-device stays in [channel, token]
#       layout so no PE transposes are needed in L2)
# ln_w / ffn_w are folded into the matmul weights on host (exact).

import sys

sys.path.insert(0, "/opt/trn_rl_repo")

import numpy as np
import ml_dtypes

import concourse.bass as bass
import concourse.tile as tile
from concourse import mybir, bacc, masks
from concourse.bass_utils import run_bass_kernel_spmd

B, L, D = 4, 8192, 1024
NCORES = 8
T = L // 2        # tokens per core
TT = 512          # token tile
NT = T // TT      # 8 token tiles per core
NSUB = TT // 128  # 4 norm sub-tiles per token tile
KC = D // 128     # contraction chunks
EC = D // 128     # output-channel chunks
EPS = 1e-6

f32 = mybir.dt.float32
bf16 = mybir.dt.bfloat16
AF = mybir.ActivationFunctionType
OP = mybir.AluOpType
bf16_np = ml_dtypes.bfloat16


def build_l1():
    nc = bacc.Bacc(None, target_bir_lowering=False)
    inp_s = nc.dram_tensor("inp_s", [T, D], f32, kind="ExternalInput")
    wgT_d = nc.dram_tensor("wgT", [D, D], bf16, kind="ExternalInput")
    wcT_d = nc.dram_tensor("wcT", [D, D], bf16, kind="ExternalInput")
    a_T = nc.dram_tensor("a_T", [D, T], bf16, kind="ExternalOutput")
    x_T = nc.dram_tensor("x_T", [D, T], bf16, kind="ExternalOutput")
    hlast = nc.dram_tensor("hlast", [128, EC], f32, kind="ExternalOutput")

    with tile.TileContext(nc) as tc:
        with (
            tc.tile_pool(name="wpool", bufs=1) as wpool,
            tc.tile_pool(name="xin", bufs=4) as xin,
            tc.tile_pool(name="norm", bufs=3) as norm,
            tc.tile_pool(name="xnt", bufs=2) as xntp,
            tc.tile_pool(name="gate", bufs=4) as gate,
            tc.tile_pool(name="scan", bufs=2) as scanp,
            tc.tile_pool(name="per", bufs=1) as per,
            tc.tile_pool(name="psum_mm", bufs=2, space=bass.MemorySpace.PSUM) as psum_mm,
            tc.tile_pool(name="psum_tr", bufs=4, space=bass.MemorySpace.PSUM) as psum_tr,
        ):
            ident = per.tile([128, 128], bf16)
            masks.make_identity(nc, ident[:])
            hprev = per.tile([128, EC], f32)
            nc.vector.memset(hprev[:], 0.0)
            eps_t = per.tile([128, 1], f32)
            nc.vector.memset(eps_t[:], EPS)

            wg_sb = wpool.tile([128, KC, D], bf16)
            nc.sync.dma_start(wg_sb[:], wgT_d[:].rearrange("(k p) e -> p k e", p=128))
            wc_sb = wpool.tile([128, KC, D], bf16)
            nc.sync.dma_start(wc_sb[:], wcT_d[:].rearrange("(k p) e -> p k e", p=128))

            for i in range(NT):
                # --- rmsnorm + transpose into [d, t] layout (bf16) ---
                xnt = xntp.tile([128, KC, TT], bf16)
                for s in range(NSUB):
                    t0 = i * TT + s * 128
                    xt = xin.tile([128, D], f32)
                    nc.sync.dma_start(xt[:], inp_s[t0 : t0 + 128, :])
                    sq = norm.tile([128, D], bf16, tag="sq")
                    ssq = norm.tile([128, 1], f32, tag="ssq")
                    nc.scalar.activation(sq[:], xt[:], AF.Square, accum_out=ssq[:])
                    rms = norm.tile([128, 1], f32, tag="rms")
                    nc.scalar.activation(
                        rms[:], ssq[:], AF.Sqrt, scale=1.0 / D, bias=eps_t[:]
                    )
                    inv = norm.tile([128, 1], f32, tag="inv")
                    nc.vector.reciprocal(inv[:], rms[:])
                    xn = norm.tile([128, D], bf16, tag="xn")
                    nc.vector.tensor_scalar_mul(xn[:], xt[:], inv[:])
                    for k in range(KC):
                        ptr = psum_tr.tile([128, 128], bf16)
                        nc.tensor.transpose(
                            ptr[:], xn[:, k * 128 : (k + 1) * 128], ident[:]
                        )
                        nc.vector.tensor_copy(
                            xnt[:, k, s * 128 : (s + 1) * 128], ptr[:]
                        )

                # --- gate matmuls + elementwise + local scan ---
                for e in range(EC):
                    pm_g = psum_mm.tile([128, TT], f32, tag="pmg")
                    for k in range(KC):
                        nc.tensor.matmul(
                            pm_g[:],
                            wg_sb[:, k, e * 128 : (e + 1) * 128],
                            xnt[:, k, :],
                            start=(k == 0),
                            stop=(k == KC - 1),
                        )
                    beta = gate.tile([128, TT], f32, tag="beta")
                    nc.scalar.activation(beta[:], pm_g[:], AF.Sigmoid)

                    pm_c = psum_mm.tile([128, TT], f32, tag="pmc")
                    for k in range(KC):
                        nc.tensor.matmul(
                            pm_c[:],
                            wc_sb[:, k, e * 128 : (e + 1) * 128],
                            xnt[:, k, :],
                            start=(k == 0),
                            stop=(k == KC - 1),
                        )
                    at = gate.tile([128, TT], bf16, tag="at")
                    nc.vector.tensor_scalar(at[:], beta[:], -1.0, 1.0, OP.mult, OP.add)
                    xv = gate.tile([128, TT], bf16, tag="xv")
                    nc.vector.tensor_mul(xv[:], beta[:], pm_c[:])
                    nc.sync.dma_start(
                        a_T[e * 128 : (e + 1) * 128, i * TT : (i + 1) * TT], at[:]
                    )
                    nc.sync.dma_start(
                        x_T[e * 128 : (e + 1) * 128, i * TT : (i + 1) * TT], xv[:]
                    )
                    h = scanp.tile([128, TT], f32)
                    nc.vector.tensor_tensor_scan(
                        h[:], at[:], xv[:], hprev[:, e : e + 1], OP.mult, OP.add
                    )
                    nc.vector.tensor_copy(hprev[:, e : e + 1], h[:, TT - 1 : TT])

            nc.sync.dma_start(hlast[:], hprev[:])

    nc.compile()
    return nc


def build_l2():
    nc = bacc.Bacc(None, target_bir_lowering=False)
    a_in = nc.dram_tensor("a_in", [D, T], bf16, kind="ExternalInput")
    x_in = nc.dram_tensor("x_in", [D, T], bf16, kind="ExternalInput")
    carry_d = nc.dram_tensor("carry", [128, EC], f32, kind="ExternalInput")
    w1T_d = nc.dram_tensor("w1T", [D, D], bf16, kind="ExternalInput")
    w3T_d = nc.dram_tensor("w3T", [D, D], bf16, kind="ExternalInput")
    w2T_d = nc.dram_tensor("w2T", [D, D], bf16, kind="ExternalInput")
    out_sT = nc.dram_tensor("out_sT", [D, T], f32, kind="ExternalOutput")
    hx_sT = nc.dram_tensor("hx_sT", [D, T], f32, kind="ExternalOutput")

    with tile.TileContext(nc) as tc:
        with (
            tc.tile_pool(name="wpool", bufs=1) as wpool,
            tc.tile_pool(name="ax", bufs=4) as ax,
            tc.tile_pool(name="scan", bufs=2) as scanp,
            tc.tile_pool(name="nrm", bufs=2) as nrm,
            tc.tile_pool(name="hn", bufs=2) as hnp,
            tc.tile_pool(name="ffn", bufs=2) as ffn,
            tc.tile_pool(name="outp", bufs=4) as outp,
            tc.tile_pool(name="per", bufs=1) as per,
            tc.tile_pool(name="psum_mm", bufs=2, space=bass.MemorySpace.PSUM) as psum_mm,
            tc.tile_pool(name="psum_s", bufs=2, space=bass.MemorySpace.PSUM) as psum_s,
            tc.tile_pool(name="psum_o", bufs=2, space=bass.MemorySpace.PSUM) as psum_o,
        ):
            hprev = per.tile([128, EC], f32)
            carry_sb = per.tile([128, EC], f32)
            nc.sync.dma_start(carry_sb[:], carry_d[:])
            eps_row = per.tile([1, 1], f32)
            nc.vector.memset(eps_row[:], EPS)
            ones_b = per.tile([128, 1], bf16)
            nc.vector.memset(ones_b[:], 1.0)

            w1_sb = wpool.tile([128, KC, D], bf16)
            nc.sync.dma_start(w1_sb[:], w1T_d[:].rearrange("(k p) e -> p k e", p=128))
            w3_sb = wpool.tile([128, KC, D], bf16)
            nc.sync.dma_start(w3_sb[:], w3T_d[:].rearrange("(k p) e -> p k e", p=128))
            w2_sb = wpool.tile([128, KC, D], bf16)
            nc.sync.dma_start(w2_sb[:], w2T_d[:].rearrange("(k p) e -> p k e", p=128))

            for i in range(NT):
                # --- scan + hx store + sum-of-squares accumulation (PE) ---
                hs = []
                pm_ssq = psum_s.tile([1, TT], f32)
                for c in range(EC):
                    at = ax.tile([128, TT], bf16, tag="a")
                    nc.sync.dma_start(
                        at[:], a_in[c * 128 : (c + 1) * 128, i * TT : (i + 1) * TT]
                    )
                    xv = ax.tile([128, TT], bf16, tag="x")
                    nc.sync.dma_start(
                        xv[:], x_in[c * 128 : (c + 1) * 128, i * TT : (i + 1) * TT]
                    )
                    h = scanp.tile([128, TT], f32, tag=f"h{c}")
                    init = carry_sb[:, c : c + 1] if i == 0 else hprev[:, c : c + 1]
                    nc.vector.tensor_tensor_scan(
                        h[:], at[:], xv[:], init, OP.mult, OP.add
                    )
                    nc.vector.tensor_copy(hprev[:, c : c + 1], h[:, TT - 1 : TT])
                    nc.sync.dma_start(
                        hx_sT[c * 128 : (c + 1) * 128, i * TT : (i + 1) * TT], h[:]
                    )
                    hsq = nrm.tile([128, TT], bf16, tag="hsq")
                    nc.scalar.square(hsq[:], h[:])
                    nc.tensor.matmul(
                        pm_ssq[:],
                        ones_b[:],
                        hsq[:],
                        start=(c == 0),
                        stop=(c == EC - 1),
                    )
                    hs.append(h)

                # --- rms row -> inv -> broadcast -> hn^T (bf16) ---
                rms_row = nrm.tile([1, TT], f32, tag="rmsrow")
                nc.scalar.activation(
                    rms_row[:], pm_ssq[:], AF.Sqrt, scale=1.0 / D, bias=eps_row[:]
                )
                inv_row = nrm.tile([1, TT], f32, tag="invrow")
                nc.vector.reciprocal(inv_row[:], rms_row[:])
                inv_bc = nrm.tile([128, TT], f32, tag="invbc")
                nc.gpsimd.partition_broadcast(inv_bc[:], inv_row[:])
                hnT = []
                for c in range(EC):
                    hn_c = hnp.tile([128, TT], bf16, tag=f"hn{c}")
                    nc.vector.tensor_mul(hn_c[:], hs[c][:], inv_bc[:])
                    hnT.append(hn_c)

                # --- FFN: u = silu(w1 @ hn) * (w3 @ hn) ---
                us = []
                for e in range(EC):
                    pm_g = psum_mm.tile([128, TT], f32, tag="pmg")
                    for k in range(KC):
                        nc.tensor.matmul(
                            pm_g[:],
                            w1_sb[:, k, e * 128 : (e + 1) * 128],
                            hnT[k][:],
                            start=(k == 0),
                            stop=(k == KC - 1),
                        )
                    sil = ffn.tile([128, TT], bf16, tag="sil")
                    nc.scalar.activation(sil[:], pm_g[:], AF.Silu)
                    pm_r = psum_mm.tile([128, TT], f32, tag="pmr")
                    for k in range(KC):
                        nc.tensor.matmul(
                            pm_r[:],
                            w3_sb[:, k, e * 128 : (e + 1) * 128],
                            hnT[k][:],
                            start=(k == 0),
                            stop=(k == KC - 1),
                        )
                    u_e = ffn.tile([128, TT], bf16, tag=f"u{e}")
                    nc.vector.tensor_mul(u_e[:], sil[:], pm_r[:])
                    us.append(u_e)

                # --- ff^T = w2 @ u, residual in [d, t], store ---
                for e in range(EC):
                    pm_f = psum_o.tile([128, TT], f32)
                    for hc in range(KC):
                        nc.tensor.matmul(
                            pm_f[:],
                            w2_sb[:, hc, e * 128 : (e + 1) * 128],
                            us[hc][:],
                            start=(hc == 0),
                            stop=(hc == KC - 1),
                        )
                    outf = outp.tile([128, TT], f32)
                    nc.vector.tensor_add(outf[:], pm_f[:], hs[e][:])
                    nc.sync.dma_start(
                        out_sT[e * 128 : (e + 1) * 128, i * TT : (i + 1) * TT], outf[:]
                    )

    nc.compile()
    return nc


_CACHE = {}
last_perf = {}


def _get_programs():
    if "l1" not in _CACHE:
        _CACHE["l1"] = build_l1()
        _CACHE["l2"] = build_l2()
    return _CACHE["l1"], _CACHE["l2"]


def kernel(inp, Wg, Wc, w1, w2, w3, ln_w, ffn_w):
    import os
    import time

    trace = bool(int(os.environ.get("MINGRU_TRACE", "0")))
    nc1, nc2 = _get_programs()

    inp = np.asarray(inp, np.float32)
    # fold the norm scales into the following matmuls (exact)
    wgT = np.ascontiguousarray((np.asarray(Wg, np.float32) * np.asarray(ln_w, np.float32)).T).astype(bf16_np)
    wcT = np.ascontiguousarray((np.asarray(Wc, np.float32) * np.asarray(ln_w, np.float32)).T).astype(bf16_np)
    w1T = np.ascontiguousarray((np.asarray(w1, np.float32) * np.asarray(ffn_w, np.float32)).T).astype(bf16_np)
    w3T = np.ascontiguousarray((np.asarray(w3, np.float32) * np.asarray(ffn_w, np.float32)).T).astype(bf16_np)
    w2T = np.ascontiguousarray(np.asarray(w2, np.float32).T).astype(bf16_np)

    in1 = []
    for c in range(NCORES):
        b, half = divmod(c, 2)
        in1.append(
            {
                "inp_s": np.ascontiguousarray(inp[b, half * T : (half + 1) * T, :]),
                "wgT": wgT,
                "wcT": wcT,
            }
        )
    t0 = time.time()
    r1 = run_bass_kernel_spmd(nc1, in1, core_ids=list(range(NCORES)), trace=trace)
    t1 = time.time()

    zeros = np.zeros((128, EC), np.float32)
    in2 = []
    for c in range(NCORES):
        b, half = divmod(c, 2)
        carry = r1.results[2 * b]["hlast"] if half == 1 else zeros
        in2.append(
            {
                "a_in": r1.results[c]["a_T"],
                "x_in": r1.results[c]["x_T"],
                "carry": np.ascontiguousarray(carry),
                "w1T": w1T,
                "w3T": w3T,
                "w2T": w2T,
            }
        )
    t2 = time.time()
    r2 = run_bass_kernel_spmd(nc2, in2, core_ids=list(range(NCORES)), trace=trace)
    t3 = time.time()

    out = np.empty((B, L, D), np.float32)
    hx = np.empty((B, L, D), np.float32)
    for c in range(NCORES):
        b, half = divmod(c, 2)
        out[b, half * T : (half + 1) * T, :] = r2.results[c]["out_sT"].T
        hx[b, half * T : (half + 1) * T, :] = r2.results[c]["hx_sT"].T

    last_perf["r1"] = r1
    last_perf["r2"] = r2
    last_perf["t_l1"] = t1 - t0
    last_perf["t_l2"] = t3 - t2
    return out, hx
